# revision 47
# baseline (speedup 1.0000x reference)
"""nn_DecoderBlock Trainium2 kernel — 8 NeuronCores, token-sharded.

Self-contained: builds a Bass/Tile SPMD program (one program, all 8
cores; per-core differences are input data), runs it via
run_bass_kernel_spmd, reassembles the full output on the host.
"""



import math
from contextlib import ExitStack

import numpy as np
import ml_dtypes

import concourse.bass as bass
import concourse.mybir as mybir
from concourse.tile import TileContext
from concourse.masks import make_identity

try:
    from tile_patch import split_excess_waits
except ImportError:  # self-contained kernel.py defines it later in-file
    pass

F32 = mybir.dt.float32
BF16 = mybir.dt.bfloat16
F8 = mybir.dt.float8e4
AF = mybir.ActivationFunctionType
ALU = mybir.AluOpType
AX = mybir.AxisListType

NEG = -1.0e9
CORES = 8
GPC = 4


def full_cfg():
    return dict(B=2, T=2048, D=2048, H=16, DFF=4096)


def small_cfg():
    return dict(B=2, T=1024, D=512, H=4, DFF=1024)


def derived(cfg):
    B, T, D, H, DFF = cfg["B"], cfg["T"], cfg["D"], cfg["H"], cfg["DFF"]
    HD = D // H
    assert HD == 128
    TOK = B * T // CORES
    assert T // GPC == TOK and TOK % 128 == 0
    return dict(HD=HD, TOK=TOK, NT=TOK // 128, KD=D // 128, KF=DFF // 128,
                NKB=T // 128)


# Balanced causal sharding: core p (within its 4-core group) owns query
# blocks [p, 7-p, 8+p, 15-p] — ascending causal depth, equal total work on
# every core. Slot j (0..3) has static bound 4(j+1) key blocks.
def own_blocks(p, nt):
    # generalized for NT=2 (small cfg): [p, 2*GPC-1-p, ...] pattern
    out = []
    for j in range(nt):
        base = j // 2 * 2 * GPC
        out.append(base + (p if j % 2 == 0 else 2 * GPC - 1 - p))
    return out


def kb_rank_slot(b, nt):
    # inverse of own_blocks: which (rank, slot) owns global block b
    base = (b // (2 * GPC)) * 2 * GPC
    r = b - base
    if r < GPC:
        return r, (b // (2 * GPC)) * 2
    return 2 * GPC - 1 - r, (b // (2 * GPC)) * 2 + 1


def attn_packs(nt, nkb):
    # pack score blocks into 2-bank (1024-col) PSUM tiles, exactly filling
    # 512-col banks; width of block kb = q columns of slots with bound > kb
    w = {kb: 128 * nt - (kb // GPC) * 128 for kb in range(nkb)}
    packs, cur, off = [], [], 0
    # order: pair full-width blocks, then (w384,w128) pairs, then w256 pairs
    order = []
    groups = {}
    for kb in range(nkb):
        groups.setdefault(w[kb], []).append(kb)
    ws = sorted(groups, reverse=True)
    # greedy fill banks of 512 with (w, 512-w) partners
    bank_fill = []
    used = set()
    for wa in ws:
        for kb in groups[wa]:
            if kb in used:
                continue
            used.add(kb)
            fill = [kb]
            rem = 512 - wa
            while rem > 0:
                cand = None
                for wb in ws:
                    if wb <= rem and any(k2 not in used for k2 in groups[wb]):
                        cand = next(k2 for k2 in groups[wb] if k2 not in used)
                        break
                if cand is None:
                    break
                used.add(cand)
                fill.append(cand)
                rem -= w[cand]
            bank_fill.append(fill)
    # two banks per pack tile
    for i in range(0, len(bank_fill), 2):
        pack = []
        off = 0
        for bi, fill in enumerate(bank_fill[i:i + 2]):
            off = bi * 512
            for kb in fill:
                pack.append((kb, off))
                off += w[kb]
        packs.append(pack)
    return packs, w


def build(nc: bass.Bass, cfg):
    B, T, D, H, DFF = cfg["B"], cfg["T"], cfg["D"], cfg["H"], cfg["DFF"]
    dv = derived(cfg)
    TOK, NT, KD, KF, NKB = dv["TOK"], dv["NT"], dv["KD"], dv["KF"], dv["NKB"]
    DCH = min(512, D)
    NDC = D // DCH
    RMS_EPS = float(np.finfo(np.float32).eps)
    LN_EPS = 1e-5
    DT = D * TOK

    x_in = nc.declare_dram_parameter("x", [TOK, D], F32, isOutput=False)
    wq = nc.declare_dram_parameter("wq", [D, D], BF16, isOutput=False)
    wk = nc.declare_dram_parameter("wk", [D, D], BF16, isOutput=False)
    wv = nc.declare_dram_parameter("wv", [D, D], BF16, isOutput=False)
    wo = nc.declare_dram_parameter("wo", [(D // 256) * (D // 512) * 128, 1024],
                                   F8, isOutput=False)
    # FFN weights in fp8 (e4m3), pre-interleaved on the host for DoubleRow:
    # row-block index = (j * nmb + mb) * 128, each [128, 1024] tile holding
    # [ki, (ko, n)] with contraction d = (2j + ko) * 128 + ki.
    w1 = nc.declare_dram_parameter("w1", [(D // 256) * (DFF // 512) * 128, 1024],
                                   F8, isOutput=False)
    wg1 = nc.declare_dram_parameter("wg1", [(DFF // 256) * (DFF // 512) * 128, 1024],
                                    F8, isOutput=False)
    wg2 = nc.declare_dram_parameter("wg2", [(DFF // 256) * (DFF // 512) * 128, 1024],
                                    F8, isOutput=False)
    w2 = nc.declare_dram_parameter("w2", [(DFF // 256) * (D // 512) * 128, 1024],
                                   F8, isOutput=False)
    bqc_d = nc.declare_dram_parameter("bqc", [D], F32, isOutput=False)
    bkp_d = nc.declare_dram_parameter("bkp", [D], F32, isOutput=False)
    b1_d = nc.declare_dram_parameter("b1p", [DFF], F32, isOutput=False)
    bg1_d = nc.declare_dram_parameter("bg1", [DFF], F32, isOutput=False)
    bg2_d = nc.declare_dram_parameter("bg2", [DFF], F32, isOutput=False)
    bo_rep_d = nc.declare_dram_parameter("bo_rep", [128, D], F32, isOutput=False)
    b2_rep_d = nc.declare_dram_parameter("b2_rep", [128, D], F32, isOutput=False)
    cos_d = nc.declare_dram_parameter("cosT", [128, TOK], F32, isOutput=False)
    sin_d = nc.declare_dram_parameter("sinT", [128, TOK], F32, isOutput=False)
    kbown_d = nc.declare_dram_parameter("keybias_own", [TOK], F32, isOutput=False)
    validrep_d = nc.declare_dram_parameter("validrep", [128, NKB * 128], BF16,
                                           isOutput=False)
    PACKS, KBW = attn_packs(NT, NKB)
    pack_w = [max(off + KBW[kb] for kb, off in pk) for pk in PACKS]
    MSK_W = sum(pack_w)
    mask2d_d = nc.declare_dram_parameter("mask2d", [128, MSK_W], BF16,
                                         isOutput=False)
    out_d = nc.declare_dram_parameter("out", [TOK, D], F32, isOutput=True)

    with TileContext(nc) as tc, ExitStack() as top:
        constp = top.enter_context(tc.tile_pool(name="constp", bufs=1))
        dramp = top.enter_context(tc.tile_pool(name="dramp", bufs=1, space="DRAM"))
        wsp = top.enter_context(tc.tile_pool(name="wsp", bufs=16))
        x2p = top.enter_context(tc.tile_pool(name="x2p", bufs=1))

        # ---- constants
        ident = constp.tile([128, 128], BF16, name="ident")
        make_identity(nc, ident[:])
        cosT = constp.tile([128, TOK], F32, name="cosT")
        sinT = constp.tile([128, TOK], F32, name="sinT")
        nc.sync.dma_start(cosT[:], cos_d[:])
        nc.sync.dma_start(sinT[:], sin_d[:])
        # valid_rep[:, kb*128:(kb+1)*128]: 128 identical columns holding the
        # pad-valid indicator for gathered key block kb (softmax denominator
        # weights — padded keys contribute nothing).
        valid_rep = constp.tile([128, NKB * 128], BF16, name="valid_rep")
        nc.sync.dma_start(valid_rep[:], validrep_d[:])
        # additive score mask tiles, one per PSUM pack (causal NEG / diagonal
        # tri / zero), host-built per core
        mask2d = constp.tile([128, MSK_W], BF16, name="mask2d")
        nc.sync.dma_start(mask2d[:], mask2d_d[:])
        # vscale[:, t]: 1/0 pad indicator for own token block t (zeroes v rows)
        vscale = constp.tile([128, NT], F32, name="vscale")
        nc.sync.dma_start(vscale[:], kbown_d[:].rearrange("(n p) -> p n", p=128))
        bqc = constp.tile([128, KD], F32, name="bqc")
        nc.sync.dma_start(bqc[:], bqc_d[:].rearrange("(n p) -> p n", p=128))
        bkp = constp.tile([128, KD], F32, name="bkp")
        nc.sync.dma_start(bkp[:], bkp_d[:].rearrange("(n p) -> p n", p=128))
        b1t = constp.tile([128, KF], F32, name="b1t")
        nc.sync.dma_start(b1t[:], b1_d[:].rearrange("(n p) -> p n", p=128))
        bg1t = constp.tile([128, KF], F32, name="bg1t")
        nc.sync.dma_start(bg1t[:], bg1_d[:].rearrange("(n p) -> p n", p=128))
        bg2t = constp.tile([128, KF], F32, name="bg2t")
        nc.sync.dma_start(bg2t[:], bg2_d[:].rearrange("(n p) -> p n", p=128))
        bo_rep = constp.tile([128, D], F32, name="bo_rep")
        nc.sync.dma_start(bo_rep[:], bo_rep_d[:])
        b2_rep = constp.tile([128, D], F32, name="b2_rep")
        nc.sync.dma_start(b2_rep[:], b2_rep_d[:])

        snd_k = [dramp.tile([DT // 2], F8, name=f"snd_k{i}") for i in (0, 1)]
        snd_v = [dramp.tile([DT // 2], F8, name=f"snd_v{i}") for i in (0, 1)]
        gat_k = [dramp.tile([GPC, DT // 2], F8, name=f"gat_k{i}")
                 for i in (0, 1)]
        gat_v = [dramp.tile([GPC, DT // 2], F8, name=f"gat_v{i}")
                 for i in (0, 1)]

        x2_t = [x2p.tile([128, D], F32, name=f"x2_{t}") for t in range(NT)]
        sums_x2 = [x2p.tile([128, 1], F32, name=f"sx2_{t}") for t in range(NT)]
        h2T8 = [x2p.tile([128, 2 * TOK], F8, name=f"h2T8_{j}")
                for j in range(KD // 2)]

        def pv(ap):
            # [128, 1024] fp8 pair tile -> [128, 2, 512] DoubleRow AP
            return ap.rearrange("p (two n) -> p two n", two=2)

        DR = mybir.MatmulPerfMode.DoubleRow

        with tc.tile_pool(name="ctxp", bufs=1) as ctxp:
            ctxT8 = [ctxp.tile([128, 2 * TOK], F8, name=f"ctxT8_{j}")
                     for j in range(H // 2)]

            with tc.tile_pool(name="hTp", bufs=1) as hTp:
                hT = [hTp.tile([128, TOK], BF16, name=f"hT_{k}")
                      for k in range(KD)]

                # ===== phase 1: RMSNorm + transpose -> hT
                with tc.tile_pool(name="ph1w", bufs=2) as ph1w, \
                     tc.tile_pool(name="ps1", bufs=4, space="PSUM") as ps1:
                    for t in range(NT):
                        xt = ph1w.tile([128, D], F32, name="xt", tag="xt")
                        # scalar-engine queue: keeps the x loads off the
                        # weight-stream queue so phase 1 starts immediately
                        nc.sync.dma_start(xt[:], x_in[t * 128:(t + 1) * 128, :])
                        sq = ph1w.tile([128, D], F32, name="sq", tag="sq")
                        ssum = ph1w.tile([128, 1], F32, name="ssum", tag="ssum")
                        nc.scalar.activation(sq[:], xt[:], AF.Square,
                                             accum_out=ssum[:])
                        nc.vector.tensor_scalar(
                            ssum[:], ssum[:], 1.0 / D, RMS_EPS,
                            op0=ALU.mult, op1=ALU.add)
                        nc.scalar.sqrt(ssum[:], ssum[:])
                        rs = ph1w.tile([128, 1], F32, name="rs", tag="rs")
                        nc.vector.reciprocal(rs[:], ssum[:])
                        hn = ph1w.tile([128, D], BF16, name="hn",
                                       tag="hn", bufs=2)
                        nc.scalar.activation(hn[:], xt[:], AF.Copy, scale=rs[:])
                        for k in range(KD):
                            tp = ps1.tile([128, 128], BF16, name="tp", tag="tp")
                            nc.tensor.transpose(
                                tp[:], hn[:, k * 128:(k + 1) * 128], ident[:])
                            nc.vector.tensor_copy(
                                hT[k][:, t * 128:(t + 1) * 128], tp[:])

                with tc.tile_pool(name="qkvp", bufs=1) as qkvp:
                    qrT = [qkvp.tile([128, TOK], BF16, name=f"qrT_{k}")
                           for k in range(KD)]
                    krT = [qkvp.tile([128, TOK], F8, name=f"krT_{k}")
                           for k in range(KD)]
                    vtok = [qkvp.tile([128, D], F8, name=f"vtok_{t}")
                            for t in range(NT)]

                    # ===== phase 2: projections + rope + send + gather
                    with tc.tile_pool(name="ph2w", bufs=4) as ph2w, \
                         tc.tile_pool(name="ps2", bufs=2, space="PSUM") as ps2:

                        def rope(dst, src):
                            # walrus: SB+SB tensor_tensor operands must share
                            # base partition -> cos/sin are replicated on both
                            # halves and tmps live at base 0
                            t1 = ph2w.tile([64, TOK], F32, name="rp1", tag="rp1")
                            t2 = ph2w.tile([64, TOK], F32, name="rp2", tag="rp2")
                            t3 = ph2w.tile([64, TOK], F32, name="rp3", tag="rp3")
                            t4 = ph2w.tile([64, TOK], F32, name="rp4", tag="rp4")
                            nc.vector.tensor_mul(t1[:], src[0:64, :], cosT[0:64, :])
                            nc.vector.tensor_mul(t2[:], src[64:128, :], sinT[64:128, :])
                            nc.vector.tensor_sub(dst[0:64, :], t1[:], t2[:])
                            nc.vector.tensor_mul(t3[:], src[0:64, :], sinT[0:64, :])
                            nc.vector.tensor_mul(t4[:], src[64:128, :], cosT[64:128, :])
                            nc.vector.tensor_add(dst[64:128, :], t3[:], t4[:])

                        qscale = 1.0 / math.sqrt(128.0)

                        def proj_fmajor(wten, bias_t, scale_, dstl, send):
                            for mb in range(KD // 4):
                                psl = [ps2.tile([128, DCH], F32, name=f"mm{m}",
                                                tag=f"mm{m}") for m in range(4)]
                                for k in range(KD):
                                    wt = wsp.tile([128, 512], BF16, name="wt",
                                                  tag="w")
                                    nc.sync.dma_start(
                                        wt[:], wten[k * 128:(k + 1) * 128,
                                                    mb * 512:(mb + 1) * 512])
                                    for m in range(4):
                                        nc.tensor.matmul(
                                            psl[m][:, 0:TOK],
                                            wt[:, m * 128:(m + 1) * 128],
                                            hT[k][:], start=(k == 0),
                                            stop=(k == KD - 1))
                                for m in range(4):
                                    kd = mb * 4 + m
                                    raw = ph2w.tile([128, TOK], BF16,
                                                    name="rawqk", tag="rawqk")
                                    nc.scalar.activation(
                                        raw[:], psl[m][:, 0:TOK], AF.Identity,
                                        bias=bias_t[:, kd:kd + 1], scale=scale_)
                                    rope(dstl[kd][:], raw[:])
                                    if send:
                                        hk = kd // (KD // 2)
                                        kk = kd % (KD // 2)
                                        nc.scalar.dma_start(
                                            snd_k[hk][kk * 128 * TOK:
                                                      (kk + 1) * 128 * TOK]
                                            .rearrange("(p t) -> p t", t=TOK),
                                            dstl[kd][:])
                                if send and mb == KD // 8 - 1:
                                    # first half of k heads sent: gather now
                                    nc.gpsimd.collective_compute(
                                        "AllGather", ALU.bypass,
                                        replica_groups=[[0, 1, 2, 3],
                                                        [4, 5, 6, 7]],
                                        ins=[snd_k[0][:]], outs=[gat_k[0][:]])

                        # k first: its gathers start while v and q compute
                        proj_fmajor(wk, bkp, 1.0, krT, True)
                        nc.gpsimd.collective_compute(
                            "AllGather", ALU.bypass,
                            replica_groups=[[0, 1, 2, 3], [4, 5, 6, 7]],
                            ins=[snd_k[1][:]], outs=[gat_k[1][:]])

                        # v token-major; each d-half gathers as it completes
                        for nd in range(NDC):
                            psl = [ps2.tile([128, DCH], F32, name=f"mm{t}",
                                            tag=f"mm{t}") for t in range(NT)]
                            for k in range(KD):
                                wt = wsp.tile([128, 512], BF16, name="wt",
                                              tag="w")
                                nc.sync.dma_start(
                                    wt[:], wv[k * 128:(k + 1) * 128,
                                              nd * 512:(nd + 1) * 512])
                                for t in range(NT):
                                    nc.tensor.matmul(
                                        psl[t][:],
                                        hT[k][:, t * 128:(t + 1) * 128], wt[:],
                                        start=(k == 0), stop=(k == KD - 1))
                            for t in range(NT):
                                # pad-zeroed v rows: padded keys then drop out
                                # of both the AV sum and the valid-weighted l
                                nc.scalar.activation(
                                    vtok[t][:, nd * 512:(nd + 1) * 512],
                                    psl[t][:], AF.Identity,
                                    scale=vscale[:, t:t + 1])
                            if nd == NDC // 2 - 1 or nd == NDC - 1:
                                hv = 0 if nd == NDC // 2 - 1 else 1
                                D2 = D // 2
                                for t in range(NT):
                                    nc.scalar.dma_start(
                                        snd_v[hv][:]
                                        .rearrange("(a d) -> a d", d=D2)
                                        [t * 128:(t + 1) * 128, :],
                                        vtok[t][:, hv * D2:(hv + 1) * D2])
                                nc.gpsimd.collective_compute(
                                    "AllGather", ALU.bypass,
                                    replica_groups=[[0, 1, 2, 3],
                                                    [4, 5, 6, 7]],
                                    ins=[snd_v[hv][:]], outs=[gat_v[hv][:]])

                        # q last: overlaps the gathers
                        proj_fmajor(wq, bqc, qscale, qrT, False)

                    # ===== phase 3: attention — unified balanced-causal loop.
                    # Every core owns query blocks of equal total causal
                    # depth; all 16 heads iterate the same packed score
                    # tiles. Causal/diagonal masking is an additive host-
                    # built mask; pad masking is zeroed v rows plus a
                    # pad-weighted denominator.
                    with tc.tile_pool(name="ph3w", bufs=3) as ph3w, \
                         tc.tile_pool(name="ps3", bufs=1, space="PSUM") as ps3:
                        QW = 128 * NT
                        PKW = max(pack_w)
                        lastpk = (len(PACKS) - 1, len(PACKS[-1]) - 1)
                        for h in range(H):
                            ktbF = ph3w.tile([128, GPC * TOK], F8,
                                             name="ktbF", tag="ktbF", bufs=2)
                            vtbF = ph3w.tile([128, GPC * TOK], F8,
                                             name="vtbF", tag="vtbF", bufs=2)
                            hh = h // (H // 2)
                            ho = h - hh * (H // 2)
                            D2 = D // 2
                            for r in range(GPC):
                                nc.scalar.dma_start(
                                    ktbF[:, r * TOK:(r + 1) * TOK],
                                    gat_k[hh][r, :]
                                    .rearrange("(d t) -> d t", t=TOK)
                                    [ho * 128:(ho + 1) * 128, :])
                                nc.scalar.dma_start(
                                    vtbF[:, r * TOK:(r + 1) * TOK]
                                    .rearrange("p (a d) -> p a d", a=NT),
                                    gat_v[hh][r, :]
                                    .rearrange("(a p d) -> p a d", p=128, d=D2)
                                    [:, :, ho * 128:(ho + 1) * 128])
                            avps = ps3.tile([128, TOK], F32, name="avps",
                                            tag="avps", bufs=2)
                            lrep = ps3.tile([128, TOK], F32, name="lrep",
                                            tag="lrep", bufs=2)
                            moff = 0
                            for ti, pack in enumerate(PACKS):
                                pw = pack_w[ti]
                                sps2 = ps3.tile([128, PKW], F32,
                                                name="sps2", tag="sps2",
                                                bufs=2)
                                # additive mask seeds each bank (start=True),
                                # qk matmuls accumulate scores on top
                                for mb0 in range(0, pw, 512):
                                    mw = min(512, pw - mb0)
                                    ents = [e for e in pack
                                            if mb0 <= e[1] < mb0 + 512]
                                    nc.tensor.matmul(
                                        sps2[:, mb0:mb0 + mw], ident[:],
                                        mask2d[:, moff + mb0:
                                               moff + mb0 + mw],
                                        start=True, stop=False,
                                        skip_group_check=True)
                                    for ei, (kb, off) in enumerate(ents):
                                        w = KBW[kb]
                                        r, s = kb_rank_slot(kb, NT)
                                        ks = r * TOK + s * 128
                                        nc.tensor.matmul(
                                            sps2[:, off:off + w],
                                            ktbF[:, ks:ks + 128],
                                            qrT[h][:, QW - w:QW],
                                            start=False,
                                            stop=ei == len(ents) - 1,
                                            skip_group_check=True)
                                p2 = ph3w.tile([128, PKW], BF16,
                                               name="p2", tag="p2", bufs=6)
                                nc.scalar.activation(p2[:, 0:pw],
                                                     sps2[:, 0:pw], AF.Exp)
                                for pi, (kb, off) in enumerate(pack):
                                    w = KBW[kb]
                                    r, s = kb_rank_slot(kb, NT)
                                    ks = r * TOK + s * 128
                                    first = ti == 0 and pi == 0
                                    last = (ti, pi) == lastpk
                                    ph = p2[:, off:off + w]
                                    nc.tensor.matmul(
                                        lrep[:, QW - w:QW],
                                        valid_rep[:, kb * 128:(kb + 1) * 128],
                                        ph, start=first, stop=last,
                                        skip_group_check=True)
                                    nc.tensor.matmul(
                                        avps[:, QW - w:QW],
                                        vtbF[:, ks:ks + 128],
                                        ph, start=first, stop=last,
                                        skip_group_check=True)
                                moff += pw
                            linv = ph3w.tile([128, TOK], F32, name="linv",
                                             tag="linv", bufs=2)
                            nc.vector.reciprocal(linv[:], lrep[:])
                            half = (h % 2) * TOK
                            nc.vector.tensor_mul(
                                ctxT8[h // 2][:, half:half + TOK],
                                avps[:], linv[:])

            # ===== phase 4+5: Wo (fp8 DoubleRow, nd-outer so each weight
            # tile streams once) + residual; LayerNorm pipelines after in the
            # same pool scope (no inter-phase barrier)
            with tc.tile_pool(name="ph45w", bufs=3) as ph45w, \
                 tc.tile_pool(name="wspo", bufs=8) as wspo, \
                 tc.tile_pool(name="ps45", bufs=1, space="PSUM") as ps45:
                NJO = KD // 2
                for nd in range(NDC):
                    psl = [ps45.tile([128, DCH], F32, name=f"mmo{t}",
                                     tag=f"mmo{t}") for t in range(NT)]
                    for j in range(NJO):
                        wt = wspo.tile([128, 1024], F8, name="wto", tag="wo8")
                        nc.sync.dma_start(
                            wt[:], wo[(j * NDC + nd) * 128:
                                      (j * NDC + nd + 1) * 128, :])
                        for t in range(NT):
                            nc.tensor.matmul(
                                psl[t][:],
                                pv(ctxT8[j][:])[:, :, t * 128:(t + 1) * 128],
                                pv(wt[:]), start=(j == 0),
                                stop=(j == NJO - 1), perf_mode=DR)
                    for t in range(NT):
                        xf = ph45w.tile([128, DCH], F32, name="xf", tag="xf")
                        nc.sync.dma_start(
                            xf[:], x_in[t * 128:(t + 1) * 128,
                                        nd * DCH:(nd + 1) * DCH])
                        tt1 = ph45w.tile([128, DCH], F32, name="tt1",
                                         tag="tt1")
                        nc.vector.tensor_add(tt1[:], psl[t][:], xf[:])
                        nc.vector.tensor_add(
                            x2_t[t][:, nd * DCH:(nd + 1) * DCH], tt1[:],
                            bo_rep[:, nd * DCH:(nd + 1) * DCH])
                for t in range(NT):
                    nc.vector.tensor_reduce(sums_x2[t][:], x2_t[t][:],
                                            axis=AX.X, op=ALU.add)
                    nmu = ph45w.tile([128, 1], F32, name="nmu", tag="nmu")
                    nc.vector.tensor_scalar(nmu[:], sums_x2[t][:], -1.0 / D,
                                            None, op0=ALU.mult)
                    sq = ph45w.tile([128, D], F32, name="sq5", tag="sq5")
                    var = ph45w.tile([128, 1], F32, name="var", tag="var")
                    nc.scalar.activation(sq[:], x2_t[t][:], AF.Square,
                                         bias=nmu[:], accum_out=var[:])
                    nc.vector.tensor_scalar(var[:], var[:], 1.0 / D, LN_EPS,
                                            op0=ALU.mult, op1=ALU.add)
                    nc.scalar.sqrt(var[:], var[:])
                    rs = ph45w.tile([128, 1], F32, name="rs5", tag="rs5")
                    nc.vector.reciprocal(rs[:], var[:])
                    nrs = ph45w.tile([128, 1], F32, name="nrs", tag="nrs")
                    nc.vector.tensor_mul(nrs[:], nmu[:], rs[:])
                    h2 = ph45w.tile([128, D], BF16, name="h2", tag="h2")
                    nc.scalar.activation(h2[:], x2_t[t][:], AF.Identity,
                                         bias=nrs[:], scale=rs[:])
                    for k in range(KD):
                        tp = ps45.tile([128, 128], BF16, name="tp5",
                                       tag="tp5", bufs=2)
                        nc.tensor.transpose(tp[:],
                                            h2[:, k * 128:(k + 1) * 128],
                                            ident[:])
                        half = (k % 2) * TOK
                        nc.vector.tensor_copy(
                            h2T8[k // 2][:, half + t * 128:
                                         half + (t + 1) * 128], tp[:])

        # ===== phases 6-7: FFN (fp8 DoubleRow), output
        with tc.tile_pool(name="ffnp", bufs=1) as ffnp, \
             tc.tile_pool(name="wsp8", bufs=16) as wsp8:
            uT8 = [ffnp.tile([128, 2 * TOK], F8, name=f"uT8_{j}")
                   for j in range(KF // 2)]
            sT8 = [ffnp.tile([128, 2 * TOK], F8, name=f"sT8_{j}")
                   for j in range(KF // 2)]

            NJ1 = KD // 2
            NJ2 = KF // 2
            NMB = KF // 4
            with tc.tile_pool(name="ph6w", bufs=2) as ph6w, \
                 tc.tile_pool(name="ps6", bufs=2, space="PSUM") as ps6:
                for mb in range(NMB):
                    psl = [ps6.tile([128, TOK], F32, name=f"mm{m}",
                                    tag=f"mm{m}") for m in range(4)]
                    for j in range(NJ1):
                        wt = wsp8.tile([128, 1024], F8, name="wt8", tag="w8")
                        nc.sync.dma_start(
                            wt[:], w1[(j * NMB + mb) * 128:
                                      (j * NMB + mb + 1) * 128, :])
                        for m in range(4):
                            nc.tensor.matmul(
                                psl[m][:], pv(wt[:])[:, :, m * 128:(m + 1) * 128],
                                pv(h2T8[j][:]), start=(j == 0),
                                stop=(j == NJ1 - 1), perf_mode=DR)
                    for m in range(4):
                        kf = mb * 4 + m
                        half = (kf % 2) * TOK
                        nc.scalar.activation(
                            uT8[kf // 2][:, half:half + TOK], psl[m][:],
                            AF.Identity, bias=b1t[:, kf:kf + 1])

                for mb in range(NMB):
                    g1l = [ph6w.tile([128, TOK], BF16, name=f"g1_{m}",
                                     tag=f"g1_{m}") for m in range(4)]
                    psl = [ps6.tile([128, TOK], F32, name=f"mm{m}",
                                    tag=f"mm{m}") for m in range(4)]
                    for j in range(NJ2):
                        wt = wsp8.tile([128, 1024], F8, name="wt8", tag="w8")
                        nc.sync.dma_start(
                            wt[:], wg1[(j * NMB + mb) * 128:
                                       (j * NMB + mb + 1) * 128, :])
                        for m in range(4):
                            nc.tensor.matmul(
                                psl[m][:], pv(wt[:])[:, :, m * 128:(m + 1) * 128],
                                pv(uT8[j][:]), start=(j == 0),
                                stop=(j == NJ2 - 1), perf_mode=DR)
                    for m in range(4):
                        kf = mb * 4 + m
                        sg = ph6w.tile([128, TOK], BF16, name="sg", tag="sg")
                        nc.scalar.activation(sg[:], psl[m][:], AF.Sigmoid,
                                             bias=bg1t[:, kf:kf + 1])
                        g1b = ph6w.tile([128, TOK], BF16, name="g1b",
                                        tag="g1b")
                        nc.scalar.activation(g1b[:], psl[m][:], AF.Identity,
                                             bias=bg1t[:, kf:kf + 1])
                        nc.vector.tensor_mul(g1l[m][:], sg[:], g1b[:])
                    psl2 = [ps6.tile([128, TOK], F32, name=f"mm{m}",
                                     tag=f"mm{m}") for m in range(4)]
                    for j in range(NJ2):
                        wt = wsp8.tile([128, 1024], F8, name="wt8", tag="w8")
                        nc.sync.dma_start(
                            wt[:], wg2[(j * NMB + mb) * 128:
                                       (j * NMB + mb + 1) * 128, :])
                        for m in range(4):
                            nc.tensor.matmul(
                                psl2[m][:], pv(wt[:])[:, :, m * 128:(m + 1) * 128],
                                pv(uT8[j][:]), start=(j == 0),
                                stop=(j == NJ2 - 1), perf_mode=DR)
                    for m in range(4):
                        kf = mb * 4 + m
                        half = (kf % 2) * TOK
                        nc.vector.scalar_tensor_tensor(
                            sT8[kf // 2][:, half:half + TOK], psl2[m][:],
                            bg2t[:, kf:kf + 1], g1l[m][:],
                            op0=ALU.add, op1=ALU.mult)

            with tc.tile_pool(name="ph7w", bufs=3) as ph7w, \
                 tc.tile_pool(name="ps7", bufs=2, space="PSUM") as ps7:
                for nd in range(NDC):
                    psl = [ps7.tile([128, DCH], F32, name=f"mm{t}",
                                    tag=f"mm{t}") for t in range(NT)]
                    for j in range(NJ2):
                        wt = wsp8.tile([128, 1024], F8, name="wt8", tag="w8")
                        nc.sync.dma_start(
                            wt[:], w2[(j * NDC + nd) * 128:
                                      (j * NDC + nd + 1) * 128, :])
                        for t in range(NT):
                            nc.tensor.matmul(
                                psl[t][:],
                                pv(sT8[j][:])[:, :, t * 128:(t + 1) * 128],
                                pv(wt[:]), start=(j == 0),
                                stop=(j == NJ2 - 1), perf_mode=DR)
                    for t in range(NT):
                        tt1 = ph7w.tile([128, DCH], F32, name="o1", tag="o1")
                        nc.vector.tensor_add(
                            tt1[:], psl[t][:],
                            x2_t[t][:, nd * DCH:(nd + 1) * DCH])
                        yf = ph7w.tile([128, DCH], F32, name="yf", tag="yf")
                        nc.vector.tensor_add(
                            yf[:], tt1[:], b2_rep[:, nd * DCH:(nd + 1) * DCH])
                        nc.sync.dma_start(
                            out_d[t * 128:(t + 1) * 128,
                                  nd * DCH:(nd + 1) * DCH], yf[:])
    n = split_excess_waits(nc)
    return nc


# ---------------------------------------------------------------- host side


def host_prepare(inputs, cfg):
    B, T, D, H, DFF = cfg["B"], cfg["T"], cfg["D"], cfg["H"], cfg["DFF"]
    dv = derived(cfg)
    HD, TOK = dv["HD"], dv["TOK"]
    f32 = np.float32
    bf = ml_dtypes.bfloat16

    x = np.asarray(inputs["x"], f32)
    g_rms = np.asarray(inputs["g_rms"], f32)
    g_ln = np.asarray(inputs["g_ln"], f32)
    b_ln = np.asarray(inputs["b_ln"], f32)
    pad = np.asarray(inputs["pad_mask"])

    perm = np.concatenate(
        [h * HD + np.concatenate([np.arange(0, HD, 2), np.arange(1, HD, 2)])
         for h in range(H)])
    wq = (g_rms[:, None] * np.asarray(inputs["Wq"], f32))[:, perm].astype(bf)
    wk = (g_rms[:, None] * np.asarray(inputs["Wk"], f32))[:, perm].astype(bf)
    wv = (g_rms[:, None] * np.asarray(inputs["Wv"], f32)).astype(bf)
    f8 = ml_dtypes.float8_e4m3fn

    def dr_pack(W):
        # [K, M] -> [(K//256)*(M//512)*128, 1024] fp8, DoubleRow interleave:
        # row (j*nmb+mb)*128+ki, col ko*512+n  <->  W[(2j+ko)*128+ki, mb*512+n]
        K, M = W.shape
        t = W.reshape(K // 256, 2, 128, M // 512, 512).transpose(0, 3, 2, 1, 4)
        return np.clip(np.ascontiguousarray(t.reshape(-1, 1024)),
                       -240.0, 240.0).astype(f8)

    wo = dr_pack(np.asarray(inputs["Wo"], f32))
    w1 = dr_pack(g_ln[:, None] * np.asarray(inputs["W1"], f32))
    wg1 = dr_pack(np.asarray(inputs["Wg1"], f32))
    wg2 = dr_pack(np.asarray(inputs["Wg2"], f32))
    w2 = dr_pack(np.asarray(inputs["W2"], f32))

    qscale = 1.0 / math.sqrt(HD)
    bqc = (np.asarray(inputs["bq"], f32)[perm] * qscale).astype(f32)
    bkp = np.asarray(inputs["bk"], f32)[perm].astype(f32)
    b1p = (np.asarray(inputs["b1"], f32)
           + b_ln @ np.asarray(inputs["W1"], f32)).astype(f32)
    bg1 = np.asarray(inputs["bg1"], f32)
    bg2 = np.asarray(inputs["bg2"], f32)
    bo_rep = np.broadcast_to(np.asarray(inputs["bo"], f32), (128, D)).copy()
    b2_rep = np.broadcast_to(np.asarray(inputs["b2"], f32), (128, D)).copy()

    inv_freq = 1.0 / (10000.0 ** (np.arange(0, HD, 2, dtype=f32) / HD))
    ang = np.arange(T, dtype=f32)[:, None] * inv_freq[None, :]
    cosA, sinA = np.cos(ang).astype(f32), np.sin(ang).astype(f32)

    NT = TOK // 128
    NKB = T // 128
    PACKS, KBW = attn_packs(NT, NKB)
    pack_w = [max(off + KBW[kb] for kb, off in pk) for pk in PACKS]
    MSK_W = sum(pack_w)
    QW = 128 * NT
    tri = np.where(np.arange(128)[:, None] <= np.arange(128)[None, :],
                   np.float32(0.0), np.float32(NEG))

    in_maps = []
    for i in range(CORES):
        g, p = i // GPC, i % GPC
        blocks = own_blocks(p, NT)
        tok_idx = np.concatenate([np.arange(b * 128, (b + 1) * 128)
                                  for b in blocks])
        # vscale: 1/0 pad indicator for own tokens (zeroes padded v rows)
        kbo = pad[g, tok_idx].astype(f32)
        # pad-valid indicator per gathered key block, replicated 128 cols
        validg = pad[g].astype(f32)
        validrep = np.broadcast_to(
            validg.reshape(NKB, 128).T[:, :, None],
            (128, NKB, 128)).reshape(128, NKB * 128).astype(bf)
        # additive causal/diagonal score mask in pack layout
        mask2d = np.zeros((128, MSK_W), f32)
        moff = 0
        for ti, pk in enumerate(PACKS):
            for kb, off in pk:
                w = KBW[kb]
                a0 = QW - w
                for j in range(a0 // 128, NT):
                    gb = blocks[j]
                    col = moff + off + j * 128 - a0
                    if kb > gb:
                        mask2d[:, col:col + 128] = NEG
                    elif kb == gb:
                        mask2d[:, col:col + 128] = tri
            moff += pack_w[ti]
        in_maps.append(dict(
            validrep=np.ascontiguousarray(validrep),
            mask2d=np.ascontiguousarray(mask2d.astype(bf)),
            x=np.ascontiguousarray(x[g, tok_idx]),
            wq=wq, wk=wk, wv=wv, wo=wo, w1=w1, wg1=wg1, wg2=wg2, w2=w2,
            bqc=bqc, bkp=bkp, b1p=b1p, bg1=bg1, bg2=bg2,
            bo_rep=bo_rep, b2_rep=b2_rep,
            cosT=np.ascontiguousarray(
                np.tile(cosA[tok_idx].T, (2, 1))),
            sinT=np.ascontiguousarray(
                np.tile(sinA[tok_idx].T, (2, 1))),
            keybias_own=kbo,
        ))
    return in_maps


def host_assemble(results, cfg):
    B, T, D = cfg["B"], cfg["T"], cfg["D"]
    TOK = derived(cfg)["TOK"]
    NT = TOK // 128
    out = np.empty((B, T, D), np.float32)
    for i in range(CORES):
        g, p = i // GPC, i % GPC
        for j, b in enumerate(own_blocks(p, NT)):
            out[g, b * 128:(b + 1) * 128] = \
                results[i]["out"][j * 128:(j + 1) * 128]
    return out


# ---------------------------------------------------------------- numpy ref


def numpy_reference(inputs, cfg):
    B, T, D, H, DFF = cfg["B"], cfg["T"], cfg["D"], cfg["H"], cfg["DFF"]
    HD = D // H
    f = np.float32
    x = np.asarray(inputs["x"], f)
    RMS_EPS = float(np.finfo(np.float32).eps)

    h = x * (1.0 / np.sqrt((x * x).mean(-1, keepdims=True) + RMS_EPS))
    h = h * inputs["g_rms"]
    q = (h @ inputs["Wq"] + inputs["bq"]).reshape(B, T, H, HD).transpose(0, 2, 1, 3)
    k = (h @ inputs["Wk"] + inputs["bk"]).reshape(B, T, H, HD).transpose(0, 2, 1, 3)
    v = (h @ inputs["Wv"]).reshape(B, T, H, HD).transpose(0, 2, 1, 3)

    inv_freq = 1.0 / (10000.0 ** (np.arange(0, HD, 2, dtype=f) / HD))
    ang = np.arange(T, dtype=f)[:, None] * inv_freq[None, :]
    cos, sin = np.cos(ang), np.sin(ang)

    def rope(z):
        z1, z2 = z[..., ::2], z[..., 1::2]
        out = np.stack([z1 * cos - z2 * sin, z1 * sin + z2 * cos], -1)
        return out.reshape(z.shape)

    q, k = rope(q), rope(k)
    scores = np.einsum("bhqd,bhkd->bhqk", q, k) / np.sqrt(np.float32(HD))
    causal = np.tril(np.ones((T, T), bool))
    mask = (np.asarray(inputs["pad_mask"])[:, None, :].astype(bool)
            & causal)[:, None]
    scores = np.where(mask, scores, -np.inf)
    m = scores.max(-1, keepdims=True)
    e = np.exp(scores - m)
    attn = e / e.sum(-1, keepdims=True)
    o = np.einsum("bhqk,bhkd->bhqd", attn, v)
    o = o.transpose(0, 2, 1, 3).reshape(B, T, D)
    x = x + o @ inputs["Wo"] + inputs["bo"]

    mu = x.mean(-1, keepdims=True)
    var = ((x - mu) ** 2).mean(-1, keepdims=True)
    h2 = (x - mu) / np.sqrt(var + 1e-5) * inputs["g_ln"] + inputs["b_ln"]
    u = h2 @ inputs["W1"] + inputs["b1"]
    g1 = u @ inputs["Wg1"] + inputs["bg1"]
    s = (g1 / (1 + np.exp(-g1))) * (u @ inputs["Wg2"] + inputs["bg2"])
    return x + s @ inputs["W2"] + inputs["b2"]


def make_small_inputs(cfg, seed=0):
    B, T, D, H, DFF = cfg["B"], cfg["T"], cfg["D"], cfg["H"], cfg["DFF"]
    rng = np.random.default_rng(seed)
    f = np.float32

    def w(shape, fan):
        return ((rng.random(shape, dtype=f) * 2 - 1) / np.sqrt(fan)).astype(f)

    lengths = rng.integers(T // 2, T + 1, size=(B,))
    pad = (np.arange(T)[None, :] < lengths[:, None]).astype(np.int32)
    return dict(
        x=rng.standard_normal((B, T, D), dtype=f),
        Wq=w((D, D), D), bq=rng.standard_normal(D, dtype=f) * 0.02,
        Wk=w((D, D), D), bk=rng.standard_normal(D, dtype=f) * 0.02,
        Wv=w((D, D), D),
        Wo=w((D, D), D), bo=rng.standard_normal(D, dtype=f) * 0.02,
        W1=w((D, DFF), D), b1=rng.standard_normal(DFF, dtype=f) * 0.02,
        Wg1=w((DFF, DFF), DFF), bg1=rng.standard_normal(DFF, dtype=f) * 0.02,
        Wg2=w((DFF, DFF), DFF), bg2=rng.standard_normal(DFF, dtype=f) * 0.02,
        W2=w((DFF, D), DFF), b2=rng.standard_normal(D, dtype=f) * 0.02,
        g_rms=(1 + 0.1 * rng.standard_normal(D)).astype(f),
        g_ln=(1 + 0.1 * rng.standard_normal(D)).astype(f),
        b_ln=(0.05 * rng.standard_normal(D)).astype(f),
        pad_mask=pad,
    )


# ===================== tile scheduler patch =====================


import concourse.tile as tile


def _split_drain_and_barrier(self, tick_clock, wait_clock):
    from concourse.vector_clock import ScopedClock

    drain_inst = self.nc.sync.drain()
    wait_clock.add_sem_waits(
        drain_inst.ins, ScopedClock({None: tick_clock.global_clock})
    )
    si = drain_inst.ins.sync_info
    waits = list(si.on_wait) if si and si.on_wait else []
    if len(waits) > 1:
        si.on_wait.clear()
        si.on_wait.extend(waits[:1])
        for i in range(1, len(waits), 1):
            extra = self.nc.sync.drain()
            esi = extra.ins.sync_info
            if esi is None:
                import concourse.mybir as mybir

                extra.ins.sync_info = mybir.SyncInfo(
                    on_wait=waits[i : i + 1], on_update=[]
                )
            else:
                esi.on_wait.extend(waits[i : i + 1])

    self.nc.all_engine_barrier()
    assert self.sems is not None
    popped = self.nc._tile_sem_poison_stack.pop()
    assert popped is self._sem_poison
    self.nc.clear_and_free_semaphores(list(self.sems.allocated().values()))
    self.nc.all_engine_barrier()


def split_excess_waits(nc, default_limit=1, ctrl_limit=1, dma_limit=1):
    """Walrus in this container rejects instructions whose sync_info
    carries more wait commands than the ISA encoding has slots for.
    Move excess waits onto same-engine no-op carriers inserted right
    before the offending instruction (engine queues are in-order, so the
    carrier's waits are observed before the instruction issues)."""
    import concourse.mybir as mybir

    CTRL = ("InstDrain", "InstNoOp", "InstEventSemaphore")
    DMA = ("InstDMACopy", "InstTriggeredCopy", "InstDMATranspose")
    nsplit = 0
    for bb_name, bbw in list(nc.bb_map.items()):
        bb = bbw.bb if hasattr(bbw, "bb") else bbw
        insts = bb.instructions
        i = 0
        while i < len(insts):
            inst = insts[i]
            tname = type(inst).__name__
            limit = (ctrl_limit if tname in CTRL
                     else dma_limit if tname in DMA else default_limit)
            si = inst.sync_info
            waits = list(si.on_wait) if si and si.on_wait else []
            if len(waits) > limit:
                keep, extra = waits[:limit], waits[limit:]
                si.on_wait.clear()
                si.on_wait.extend(keep)
                ncar = 0
                for j in range(0, len(extra), ctrl_limit):
                    chunk = extra[j:j + ctrl_limit]
                    car = nc.engines[inst.engine].nop(nofuse=True).ins
                    # nop() appended to the current bb; move it here
                    for other in nc.bb_map.values():
                        obb = other.bb if hasattr(other, "bb") else other
                        if obb.instructions and obb.instructions[-1] is car:
                            obb.instructions.pop()
                            break
                    car.sync_info = mybir.SyncInfo(on_wait=chunk, on_update=[])
                    insts.insert(i, car)
                    ncar += 1
                i += ncar
                nsplit += 1
            i += 1
    return nsplit


def _apply_tile_patch():
    tile.TileContext._drain_and_barrier = _split_drain_and_barrier


# ================================================================ runner

_tile_patch_applied = False
_build_cache = {}
LAST_EXEC_NS = None


def _get_nc():
    global _tile_patch_applied
    if not _tile_patch_applied:
        _apply_tile_patch()
        _tile_patch_applied = True
    if "nc" not in _build_cache:
        nc = bass.Bass()
        build(nc, full_cfg())
        _build_cache["nc"] = nc
    return _build_cache["nc"]


def kernel(_profile=False, **inputs):
    """Full-input decoder block on 8 TRN2 NeuronCores.

    inputs: the arrays from reference.setup_inputs() (numpy or jax).
    Returns the full [B, T, D] float32 output.
    """
    global LAST_EXEC_NS
    from concourse.bass_utils import run_bass_kernel_spmd

    cfg = full_cfg()
    nc = _get_nc()
    in_maps = host_prepare({k: np.asarray(v) for k, v in inputs.items()}, cfg)
    res = run_bass_kernel_spmd(nc, in_maps, list(range(CORES)),
                               trace=bool(_profile))
    LAST_EXEC_NS = getattr(res, "exec_time_ns", None)
    return host_assemble(res.results, cfg)

